# revision 1
# baseline (speedup 1.0000x reference)
"""Causal self-attention (B=2, T=2048, C=768, H=12, D=64) on 8 TRN2
NeuronCores via Bass/Tile, SPMD.

Sharding: core = b * 4 + hg  (batch b in {0,1}, head-group hg in {0..3},
3 heads each).  Each core computes a partial projection output
yp[b] = attn_out(heads of hg) @ W_proj[rows of hg]; the host sums the 4
partials per batch (tensor-parallel reduce done host-side since the
kernel returns the full output anyway).

Per-core on-chip layouts (chosen so no on-device transposes are needed):
  qT, kT  [64, T]  head-size on partitions; q_h and k_h of each head sit
                   at identical partition offsets (matmul base rule)
  v       [T, 64+1] natural, with a trailing all-ones column per head
  S^T     [j, i]   keys on partitions: softmax denominator comes free as
                   one extra lhsT column in the attn@v matmul, and
                   attn@v needs no transposes
  raw^T   [65, T]  rows 0-63 unnormalized attn-out^T, row 64 denominator;
                   normalization via gpsimd partition_broadcast + DVE
All matmuls run as float32r (full-rate fp32 path, moving dim >= 256).
"""

from contextlib import ExitStack

import numpy as np

import concourse.bass as bass
import concourse.mybir as mybir
import concourse.tile as tile
from concourse import bacc

F32 = mybir.dt.float32
F32R = mybir.dt.float32r

B, T, C, H, D = 2, 2048, 768, 12, 64
HL = 3            # heads per core
NCORES = 8
KC = C // 128     # 6 contraction chunks over C
NT = T // 512     # 4 chunks of t (moving dim)
TB = T // 128     # 16 blocks of t (partition dim)
SCALE = D ** -0.5


def _r(ap):
    return ap.bitcast(F32R)


def build_program(reps: int = 1) -> bacc.Bacc:
    nc = bacc.Bacc("TRN2", target_bir_lowering=False, debug=False)

    xT_d = nc.dram_tensor("xT", [C, T], F32R, kind="ExternalInput").ap()
    wqk_d = nc.dram_tensor("wqk", [C, 512], F32R, kind="ExternalInput").ap()
    wv_d = nc.dram_tensor("wv", [C, 256], F32R, kind="ExternalInput").ap()
    wp_d = nc.dram_tensor("wp", [HL * D, C], F32R, kind="ExternalInput").ap()
    yp_d = nc.dram_tensor("yp", [T, C], F32, kind="ExternalOutput").ap()

    with tile.TileContext(nc) as tc, ExitStack() as ctx:
        wpool = ctx.enter_context(tc.tile_pool(name="w", bufs=1))
        qkpool = ctx.enter_context(tc.tile_pool(name="qk", bufs=1))
        vpool = ctx.enter_context(tc.tile_pool(name="v", bufs=1))
        rawpool = ctx.enter_context(tc.tile_pool(name="raw", bufs=1))
        exppool = ctx.enter_context(tc.tile_pool(name="exp", bufs=3))
        nrmpool = ctx.enter_context(tc.tile_pool(name="nrm", bufs=1))
        ypool = ctx.enter_context(tc.tile_pool(name="y", bufs=2))
        sc_ps = ctx.enter_context(tc.tile_pool(name="scps", bufs=1, space="PSUM"))
        av_ps = ctx.enter_context(tc.tile_pool(name="avps", bufs=2, space="PSUM"))
        mm_ps = ctx.enter_context(tc.tile_pool(name="mmps", bufs=2, space="PSUM"))

        def body(_iv=None):
            # ---- load weights ----
            wqk = [wpool.tile([128, 512], F32R, tag=f"wqk{k}", name=f"wqk{k}")
                   for k in range(KC)]
            wv = [wpool.tile([128, 256], F32R, tag=f"wv{k}", name=f"wv{k}")
                  for k in range(KC)]
            wp = [wpool.tile([64, C], F32R, tag=f"wp{h}", name=f"wp{h}")
                  for h in range(HL)]
            ones_c = wpool.tile([128, 1], F32, tag="ones_c", name="ones_c")
            nc.vector.memset(ones_c[:], 1.0)
            for k in range(KC):
                nc.sync.dma_start(wqk[k][:], wqk_d[k * 128:(k + 1) * 128, :])
                nc.sync.dma_start(wv[k][:], wv_d[k * 128:(k + 1) * 128, :])
            for h in range(HL):
                nc.sync.dma_start(wp[h][:], wp_d[h * 64:(h + 1) * 64, :])

            # persistent activation tiles
            qkT = [qkpool.tile([128, T], F32, tag=f"qkT{m}", name=f"qkT{m}")
                   for m in range(4)]
            v_big = vpool.tile([128, TB * 195], F32, tag="vbig", name="v_big")
            rawT = [rawpool.tile([65, T], F32, tag=f"rawT{h}", name=f"rawT{h}")
                    for h in range(HL)]

            # tile0=(q0,q1) tile1=(k0,k1) tile2=(q2,pad) tile3=(k2,pad):
            # each head's q and k sit at the SAME partition offset.
            def qT(h):
                return qkT[0][h * 64:(h + 1) * 64, :] if h < 2 else qkT[2][0:64, :]

            def kT(h):
                return qkT[1][h * 64:(h + 1) * 64, :] if h < 2 else qkT[3][0:64, :]

            # ---- phase 1: qkT = wqk.T @ xT ; v = xT.T @ wv ----
            with tc.tile_pool(name="x", bufs=1) as xpool:
                xT = [xpool.tile([128, T], F32R, tag=f"xT{k}", name=f"xT{k}")
                      for k in range(KC)]
                for k in range(KC):
                    nc.sync.dma_start(xT[k][:], xT_d[k * 128:(k + 1) * 128, :])

                for m in range(4):
                    for t_ in range(NT):
                        ps = mm_ps.tile([128, 512], F32, tag="mm", name="ps")
                        for k in range(KC):
                            nc.tensor.matmul(
                                ps[:],
                                _r(wqk[k][:, m * 128:(m + 1) * 128]),
                                _r(xT[k][:, t_ * 512:(t_ + 1) * 512]),
                                start=(k == 0), stop=(k == KC - 1),
                            )
                        nc.scalar.copy(_r(qkT[m][:, t_ * 512:(t_ + 1) * 512]), ps[:])

                for tb in range(TB):
                    ps = mm_ps.tile([128, 512], F32, tag="mm", name="ps")
                    psv = ps[:, 0:256]
                    for k in range(KC):
                        nc.tensor.matmul(
                            psv,
                            _r(xT[k][:, tb * 128:(tb + 1) * 128]),
                            _r(wv[k][:]),
                            start=(k == 0), stop=(k == KC - 1),
                        )
                    nc.vector.tensor_copy(
                        _r(v_big[:, tb * 195:tb * 195 + 195]), ps[:, 0:195])
                    # ones column per head for the denominator trick
                    ones_view = v_big[:, tb * 195:tb * 195 + 195].rearrange(
                        "p (h c) -> p h c", h=HL)[:, :, 64:65]
                    nc.vector.tensor_copy(
                        _r(ones_view), ones_c.broadcast_to([128, HL, 1]))

            # ---- phase 2: attention per head ----
            for h in range(HL):
                for ic in range(NT):  # i chunk of 512 queries
                    av = av_ps.tile([65, 512], F32, tag="av", name="av")
                    ngroups = ic + 1
                    ets = []
                    for g in range(ngroups):  # group of 4 j-blocks
                        sps = sc_ps.tile([128, 2048], F32, tag="sc", name="sps")
                        for jj in range(4):
                            jb = 4 * g + jj
                            nc.tensor.matmul(
                                sps[:, jj * 512:(jj + 1) * 512],
                                _r(kT(h)[:, jb * 128:(jb + 1) * 128]),
                                _r(qT(h)[:, ic * 512:(ic + 1) * 512]),
                                start=True, stop=True,
                            )
                        et = exppool.tile([128, 2048], F32, tag="et", name="et")
                        nc.scalar.activation(
                            _r(et[:]), sps[:], mybir.ActivationFunctionType.Exp,
                            scale=SCALE)
                        if g == ic:
                            # diagonal: zero where j > i, i.e. keep where
                            # (col - 128*sub - row) >= 0
                            et3 = _r(et.rearrange("p (j c) -> p j c", j=4))
                            nc.gpsimd.affine_select(
                                out=et3, in_=et3,
                                compare_op=mybir.AluOpType.is_ge,
                                fill=0.0, base=0,
                                pattern=[[-128, 4], [1, 512]],
                                channel_multiplier=-1,
                            )
                        ets.append(et)
                    # attn @ v as one contiguous accumulation group
                    for g in range(ngroups):
                        for jj in range(4):
                            jb = 4 * g + jj
                            nc.tensor.matmul(
                                av[:],
                                _r(v_big[:, jb * 195 + h * 65:
                                         jb * 195 + (h + 1) * 65]),
                                _r(ets[g][:, jj * 512:(jj + 1) * 512]),
                                start=(g == 0 and jj == 0),
                                stop=(g == ngroups - 1 and jj == 3),
                            )
                    nc.vector.tensor_copy(
                        _r(rawT[h][:, ic * 512:(ic + 1) * 512]), av[:])
                # normalize rows 0..63 by the denominator in row 64.
                # partition_broadcast reads physical partition 0 on HW, so
                # first DMA the denom row to nrm's partition 0, then
                # broadcast it down (row 0 rewrites itself, same bytes).
                nrm = nrmpool.tile([64, T], F32, tag="nrm", name="nrm")
                nc.sync.dma_start(nrm[0:1, :], rawT[h][64:65, :])
                nc.gpsimd.partition_broadcast(nrm[:], nrm[0:1, :])
                nc.vector.reciprocal(nrm[:], nrm[:])
                nc.vector.tensor_mul(_r(rawT[h][0:64, :]), _r(rawT[h][0:64, :]), nrm[:])

            # ---- phase 3: yp = sum_h rawT[h].T @ wp[h] ----
            for tb in range(TB):
                yt = ypool.tile([128, C], F32, tag="yt", name="yt")
                for c0, cw in ((0, 512), (512, 256)):
                    ps = mm_ps.tile([128, 512], F32, tag="mm", name="ps")
                    psy = ps[:, 0:cw]
                    for h in range(HL):
                        nc.tensor.matmul(
                            psy,
                            _r(rawT[h][0:64, tb * 128:(tb + 1) * 128]),
                            _r(wp[h][:, c0:c0 + cw]),
                            start=(h == 0), stop=(h == HL - 1),
                        )
                    nc.scalar.copy(yt[:, c0:c0 + cw], psy)
                nc.sync.dma_start(yp_d[tb * 128:(tb + 1) * 128, :], yt[:])

        if reps == 1:
            body()
        else:
            with tc.For_i(0, reps, 1) as iv:
                body(iv)

    nc.compile()
    return nc


# ---------------- host side ----------------

def shard_inputs(x, W_qkv, W_proj):
    in_maps = []
    for core in range(NCORES):
        b, hg = divmod(core, 4)
        xT = np.ascontiguousarray(x[b].T)                      # [C, T]
        q = W_qkv[:, hg * 192: hg * 192 + 192]
        k = W_qkv[:, C + hg * 192: C + hg * 192 + 192]
        pad = np.zeros((C, 64), np.float32)
        wqk = np.ascontiguousarray(np.concatenate(
            [q[:, 0:128], k[:, 0:128], q[:, 128:192], pad, k[:, 128:192], pad],
            axis=1))
        wv = np.zeros((C, 256), np.float32)
        for h in range(HL):
            wv[:, h * 65: h * 65 + 64] = \
                W_qkv[:, 2 * C + (hg * HL + h) * 64: 2 * C + (hg * HL + h + 1) * 64]
        wp = np.ascontiguousarray(W_proj[hg * 192: hg * 192 + 192, :])
        in_maps.append({"xT": xT, "wqk": wqk, "wv": wv, "wp": wp})
    return in_maps


def unshard(results):
    y = np.zeros((B, T, C), np.float64)
    for core in range(NCORES):
        b = core // 4
        y[b] += results[core]["yp"].astype(np.float64)
    return y.astype(np.float32)


# ---------------- PJRT runner (axon-tunneled NeuronCores) ----------------

_RUNNERS = {}


def get_runner(reps: int = 1):
    """Build (once) and return fn(in_maps) -> list[dict] for 8 cores."""
    if reps in _RUNNERS:
        return _RUNNERS[reps]

    import jax
    from jax.sharding import Mesh, PartitionSpec, NamedSharding
    from jax.experimental.shard_map import shard_map
    from concourse.bass2jax import (
        _bass_exec_p, install_neuronx_cc_hook, partition_id_tensor)

    nc = build_program(reps=reps)
    install_neuronx_cc_hook()

    partition_name = nc.partition_id_tensor.name if nc.partition_id_tensor else None
    in_names, out_names, out_avals = [], [], []
    for alloc in nc.m.functions[0].allocations:
        if not isinstance(alloc, mybir.MemoryLocationSet):
            continue
        name = alloc.memorylocations[0].name
        if alloc.kind == "ExternalInput":
            if name != partition_name:
                in_names.append(name)
        elif alloc.kind == "ExternalOutput":
            out_names.append(name)
            out_avals.append(jax.core.ShapedArray(
                tuple(alloc.tensor_shape), mybir.dt.np(alloc.dtype)))
    n_params = len(in_names)
    all_in_names = in_names + out_names + ([partition_name] if partition_name else [])

    def _body(*args):
        operands = list(args)
        if partition_name is not None:
            operands.append(partition_id_tensor())
        outs = _bass_exec_p.bind(
            *operands, out_avals=tuple(out_avals), in_names=tuple(all_in_names),
            out_names=tuple(out_names), lowering_input_output_aliases=(),
            sim_require_finite=True, sim_require_nnan=True, nc=nc)
        return tuple(outs)

    devices = jax.devices()[:NCORES]
    mesh = Mesh(np.asarray(devices), ("core",))
    spec = (PartitionSpec("core"),)
    fn = jax.jit(
        shard_map(_body, mesh=mesh, in_specs=spec * (n_params + len(out_names)),
                  out_specs=spec * len(out_names), check_rep=False),
        donate_argnums=tuple(range(n_params, n_params + len(out_names))),
        keep_unused=True)
    sharding = NamedSharding(mesh, PartitionSpec("core"))

    def run(in_maps, in_dev=None):
        if in_dev is None:
            in_dev = put_inputs(in_maps, sharding, in_names)
        zeros = [
            jax.device_put(
                np.zeros((NCORES * a.shape[0], *a.shape[1:]), a.dtype), sharding)
            for a in out_avals]
        outs = fn(*in_dev, *zeros)
        return [
            {name: np.asarray(outs[i]).reshape(NCORES, *out_avals[i].shape)[c]
             for i, name in enumerate(out_names)}
            for c in range(NCORES)]

    def put_inputs(in_maps, sharding_=None, names=None):
        import jax as _jax
        sh = sharding_ or sharding
        nm = names or in_names
        return [
            _jax.device_put(
                np.concatenate([in_maps[c][n] for c in range(NCORES)], axis=0), sh)
            for n in nm]

    run.put_inputs = put_inputs
    run.sharding = sharding
    run.in_names = in_names
    _RUNNERS[reps] = run
    return run


def kernel(x, W_qkv, W_proj):
    x = np.asarray(x, dtype=np.float32)
    W_qkv = np.asarray(W_qkv, dtype=np.float32)
    W_proj = np.asarray(W_proj, dtype=np.float32)
    run = get_runner(reps=1)
    results = run(shard_inputs(x, W_qkv, W_proj))
    return unshard(results)



# revision 20
# speedup vs baseline: 1.1531x; 1.1531x over previous
"""Causal self-attention (B=2, T=2048, C=768, H=12, D=64) on 8 TRN2
NeuronCores via Bass/Tile, SPMD.

Sharding: core = b * 4 + hg  (batch b in {0,1}, head-group hg in {0..3},
3 heads each).  Each core computes a partial projection output
yp[b] = attn_out(heads of hg) @ W_proj[rows of hg]; the host sums the 4
partials per batch.

v2 (pipelined): phase 2 runs per 128-key j-block with [128,512] score
PSUM tiles, software-pipelined score -> exp -> attn@v (lag 3) so PE and
Act overlap; causal structure trims matmul/exp widths on diagonal
blocks; the only masking left is a [128,128] affine_select corner per
diagonal block (Pool).  exp output, v, attn-out and W_proj are bf16
(tolerance is 2e-2; bf16 keeps full PE rate and halves DVE/SBUF cost).
Denominators ride as a trailing ones-column in v (row 64 of the av
PSUM tile); normalization is reciprocal (DVE) -> rank-1 broadcast
matmul (PE) -> fused multiply into rawT (DVE), no SBUF-SBUF DMA and no
gpsimd broadcast.  Projection PSUM is copied to SBUF on Pool and DMA'd
out per 128-token block so stores overlap compute.

Per-core on-chip layouts:
  qT, kT  [64, T]   head-size on partitions; q_h / k_h at identical
                    partition offsets (matmul base rule)
  v_big   [128, 16*195] bf16, per key-block [v0|1|v1|1|v2|1]
  S^T     [j, i]    keys on partitions (PSUM [128,512] per j-block)
  rawT_h  [64, T]   bf16 normalized attn-out^T per head
All fp32 matmuls run as float32r (full-rate fp32, moving dim >= 256).
"""

from contextlib import ExitStack

import numpy as np

import concourse.bass as bass
import concourse.mybir as mybir
import concourse.tile as tile
from concourse import bacc

F32 = mybir.dt.float32
F32R = mybir.dt.float32r
BF16 = mybir.dt.bfloat16

B, T, C, H, D = 2, 2048, 768, 12, 64
HL = 3            # heads per core
NCORES = 8
KC = C // 128     # 6 contraction chunks over C
NT = T // 512     # 4 chunks of t (query chunks)
TB = T // 128     # 16 blocks of t
SCALE = D ** -0.5
LAG = 3           # score -> attn@v software pipeline depth (j-blocks)


def _r(ap):
    return ap.bitcast(F32R)


def build_program(reps: int = 1) -> bacc.Bacc:
    nc = bacc.Bacc("TRN2", target_bir_lowering=False, debug=False)

    xT_d = nc.dram_tensor("xT", [C, T], F32R, kind="ExternalInput").ap()
    wqk_d = nc.dram_tensor("wqk", [C, 512], F32R, kind="ExternalInput").ap()
    wv_d = nc.dram_tensor("wv", [C, 256], F32R, kind="ExternalInput").ap()
    wp_d = nc.dram_tensor("wp", [HL * D, C], BF16, kind="ExternalInput").ap()
    yp_d = nc.dram_tensor("yp", [T, C], F32, kind="ExternalOutput").ap()

    with tile.TileContext(nc) as tc, ExitStack() as ctx:
        wpool = ctx.enter_context(tc.tile_pool(name="w", bufs=1))
        qkpool = ctx.enter_context(tc.tile_pool(name="qk", bufs=1))
        vpool = ctx.enter_context(tc.tile_pool(name="v", bufs=1))
        rawpool = ctx.enter_context(tc.tile_pool(name="raw", bufs=1))
        exppool = ctx.enter_context(tc.tile_pool(name="exp", bufs=6))
        invpool = ctx.enter_context(tc.tile_pool(name="inv", bufs=2))
        ypool = ctx.enter_context(tc.tile_pool(name="y", bufs=2))
        xpool = ctx.enter_context(tc.tile_pool(name="x", bufs=1))
        mm_ps = ctx.enter_context(tc.tile_pool(name="mmps", bufs=2, space="PSUM"))
        sc_ps = ctx.enter_context(tc.tile_pool(name="scps", bufs=3, space="PSUM"))
        av_ps = ctx.enter_context(tc.tile_pool(name="avps", bufs=2, space="PSUM"))
        nr_ps = ctx.enter_context(tc.tile_pool(name="nrps", bufs=1, space="PSUM"))

        def body(_iv=None):
            # ---- weights + constants ----
            wqk = [wpool.tile([128, 512], F32R, tag=f"wqk{k}", name=f"wqk{k}")
                   for k in range(KC)]
            wv = [wpool.tile([128, 256], F32R, tag=f"wv{k}", name=f"wv{k}")
                  for k in range(KC)]
            wp = [wpool.tile([64, C], BF16, tag=f"wp{h}", name=f"wp{h}")
                  for h in range(HL)]
            ones_row = wpool.tile([65, 64], F32, tag="ones_r", name="ones_r")
            nc.vector.memset(ones_row[64:65, :], 1.0)

            # persistent activation tiles
            qkT = [qkpool.tile([128, T], F32, tag=f"qkT{m}", name=f"qkT{m}")
                   for m in range(4)]
            v_big = vpool.tile([128, TB * 195], BF16, tag="vbig", name="v_big")
            rawT = [rawpool.tile([64, T], BF16, tag=f"rawT{h}", name=f"rawT{h}")
                    for h in range(HL)]

            # ones columns of v_big (cols 64, 129, 194 of each 195-block)
            v3 = v_big.rearrange("p (t c) -> p t c", t=TB)
            for h in range(HL):
                nc.gpsimd.memset(v3[:, :, h * 65 + 64:h * 65 + 65], 1.0)

            # tile0=(q0,q1) tile1=(k0,k1) tile2=(q2,pad) tile3=(k2,pad):
            # each head's q and k sit at the SAME partition offset.
            def qT(h):
                return qkT[0][h * 64:(h + 1) * 64, :] if h < 2 else qkT[2][0:64, :]

            def kT(h):
                return qkT[1][h * 64:(h + 1) * 64, :] if h < 2 else qkT[3][0:64, :]

            # ---- phase 1: qkT = wqk.T @ xT ; v = xT.T @ wv ----
            # DMA order: serve the first t-chunk ASAP (wqk interleaved with
            # x t0 slices), then wv, then later t-chunks, then wp.
            xT = [xpool.tile([128, T], F32R, tag=f"xT{k}", name=f"xT{k}")
                  for k in range(KC)]
            for k in range(KC):
                nc.sync.dma_start(wqk[k][:], wqk_d[k * 128:(k + 1) * 128, :])
                nc.sync.dma_start(
                    xT[k][:, 0:512], xT_d[k * 128:(k + 1) * 128, 0:512])
            for k in range(KC):
                nc.sync.dma_start(wv[k][:], wv_d[k * 128:(k + 1) * 128, :])
            for t_ in range(1, NT):
                for k in range(KC):
                    nc.sync.dma_start(
                        xT[k][:, t_ * 512:(t_ + 1) * 512],
                        xT_d[k * 128:(k + 1) * 128, t_ * 512:(t_ + 1) * 512])
                if t_ == 1:
                    for h in range(HL):
                        nc.sync.dma_start(wp[h][:], wp_d[h * 64:(h + 1) * 64, :])

            # phase-1 work units, interleaved into the attention stages so
            # PE's phase-1 surplus fills Act-bound gaps
            def p1_qk(t_, m):
                sl = slice(t_ * 512, (t_ + 1) * 512)
                ps = mm_ps.tile([128, 512], F32, tag="mm", name="ps")
                for k in range(KC):
                    nc.tensor.matmul(
                        ps[:],
                        _r(wqk[k][:, m * 128:(m + 1) * 128]),
                        _r(xT[k][:, sl]),
                        start=(k == 0), stop=(k == KC - 1),
                    )
                nc.scalar.copy(_r(qkT[m][:, sl]), ps[:])

            def p1_v(t_, tb):
                ps = mm_ps.tile([128, 512], F32, tag="mm", name="psv")
                for k in range(KC):
                    nc.tensor.matmul(
                        ps[:, 0:256],
                        _r(xT[k][:, tb * 128:(tb + 1) * 128]),
                        _r(wv[k][:]),
                        start=(k == 0), stop=(k == KC - 1),
                    )
                for h in range(HL):
                    nc.vector.tensor_copy(
                        v_big[:, tb * 195 + h * 65:tb * 195 + h * 65 + 64],
                        ps[:, h * 64:(h + 1) * 64])

            def p1_units(t_):
                for m in range(4):
                    yield lambda m=m: p1_qk(t_, m)
                for tb in range(4 * t_, 4 * t_ + 4):
                    yield lambda tb=tb: p1_v(t_, tb)

            for u in p1_units(0):
                u()
            pending = []

            # ---- phase 2+3: attention pipeline ----
            # stage s = (ic, h); j-blocks jb in [0, 4*ic+4)
            # diagonal jbs (jb >= 4*ic) get restricted windows + corner mask
            def offs(ic, jb):
                if jb < 4 * ic:
                    return 0, 0          # full width (score, exp/av)
                jj = jb - 4 * ic
                return min(128 * jj, 256), 128 * jj

            stages = [(ic, h) for ic in range(NT) for h in range(HL)]
            state = {}  # s -> (av_t, et_list, invden)

            def av_mm(s, jb):
                ic, h = s
                av_t, ets, _ = state[s]
                _, e_off = offs(ic, jb)
                J = 4 * ic + 4
                nc.tensor.matmul(
                    av_t[0:65, e_off:512],
                    v_big[:, jb * 195 + h * 65:jb * 195 + (h + 1) * 65],
                    ets[jb][:, e_off:512],
                    start=(jb == 0), stop=(jb == J - 1),
                )

            def nrm_mm(s):
                nrm_t = nr_ps.tile([64, 512], F32, tag="nrm", name="nrm")
                inv = state[s][2]
                nc.tensor.matmul(
                    nrm_t[:], _r(ones_row[64:65, :]), _r(inv[64:65, :]),
                    start=True, stop=True)
                state[s] = state[s] + (nrm_t,)

            def mul(s):
                ic, h = s
                nrm_t = state[s][3]
                nc.vector.tensor_mul(
                    rawT[h][:, ic * 512:(ic + 1) * 512],
                    rawT[h][:, ic * 512:(ic + 1) * 512], nrm_t[:])
                del state[s]

            def p3_tb(tb):
                psA = mm_ps.tile([128, 512], F32, tag="mm", name="p3a")
                psB = mm_ps.tile([128, 512], F32, tag="mm", name="p3b")
                for h in range(HL):
                    nc.tensor.matmul(
                        psA[:], rawT[h][:, tb * 128:(tb + 1) * 128],
                        wp[h][:, 0:512],
                        start=(h == 0), stop=(h == HL - 1))
                for h in range(HL):
                    nc.tensor.matmul(
                        psB[:, 0:256], rawT[h][:, tb * 128:(tb + 1) * 128],
                        wp[h][:, 512:768],
                        start=(h == 0), stop=(h == HL - 1))
                yt = ypool.tile([128, C], F32, tag="yt", name="yt")
                nc.vector.tensor_copy(yt[:, 0:512], psA[:])
                nc.vector.tensor_copy(yt[:, 512:768], psB[:, 0:256])
                nc.sync.dma_start(yp_d[tb * 128:(tb + 1) * 128, :], yt[:])

            for si, s in enumerate(stages):
                ic, h = s
                J = 4 * ic + 4
                if h == 0:
                    # all of P1(ic) must be emitted before this chunk's reads
                    for u in pending:
                        u()
                    pending = list(p1_units(ic + 1)) if ic + 1 < NT else []
                av_t = av_ps.tile([128, 512], F32, tag="av", name="av")
                inv = invpool.tile([65, 512], F32, tag="inv", name="inv")
                state[s] = (av_t, [], inv)
                prev2 = stages[si - 2] if si > 1 else None
                for jb in range(J):
                    if jb >= 2 and jb % 2 == 0 and pending:
                        pending.pop(0)()
                    s_off, e_off = offs(ic, jb)
                    sps = sc_ps.tile([128, 512], F32, tag="sc", name="sps")
                    nc.tensor.matmul(
                        sps[:, s_off:512],
                        _r(kT(h)[:, jb * 128:(jb + 1) * 128]),
                        _r(qT(h)[:, ic * 512 + s_off:(ic + 1) * 512]),
                        start=True, stop=True,
                    )
                    et = exppool.tile([128, 512], BF16, tag="et", name="et")
                    nc.scalar.activation(
                        et[:, e_off:512], sps[:, e_off:512],
                        mybir.ActivationFunctionType.Exp, scale=SCALE)
                    if jb >= 4 * ic:
                        # causal corner: keep where (query - key) >= 0
                        nc.gpsimd.affine_select(
                            out=et[:, e_off:e_off + 128],
                            in_=et[:, e_off:e_off + 128],
                            compare_op=mybir.AluOpType.is_ge,
                            fill=0.0, base=0,
                            pattern=[[1, 128]],
                            channel_multiplier=-1,
                        )
                    state[s][1].append(et)
                    if jb == 1 and prev2 is not None and prev2 in state:
                        nrm_mm(prev2)
                    if jb == 2 and prev2 is not None and prev2 in state:
                        mul(prev2)
                    if 3 <= jb <= 6 and h == 1 and ic >= 1:
                        p3_tb(4 * (ic - 1) + jb - 3)
                    if jb >= LAG:
                        av_mm(s, jb - LAG)
                for r in range(max(J - LAG, 0), J):
                    av_mm(s, r)
                # unnormalized attn-out^T to SBUF now (frees the av bank);
                # normalization multiply happens after the nrm broadcast
                nc.vector.tensor_copy(
                    rawT[h][:, ic * 512:(ic + 1) * 512], av_t[0:64, :])
                with nc.allow_low_precision(reason="f32r bitcast of f32 recip"):
                    nc.vector.reciprocal(_r(inv[64:65, :]), av_t[64:65, 0:512])

            # tail: last two stages' normalization + final output chunk
            for s in stages[-2:]:
                if s in state:
                    nrm_mm(s)
                    mul(s)
            for tb in range(12, 16):
                p3_tb(tb)

        if reps == 1:
            body()
        else:
            with tc.For_i(0, reps, 1) as iv:
                body(iv)

    nc.compile()
    return nc


# ---------------- host side ----------------

def shard_inputs(x, W_qkv, W_proj):
    import ml_dtypes
    in_maps = []
    for core in range(NCORES):
        b, hg = divmod(core, 4)
        xT = np.ascontiguousarray(x[b].T)                      # [C, T]
        q = W_qkv[:, hg * 192: hg * 192 + 192]
        k = W_qkv[:, C + hg * 192: C + hg * 192 + 192]
        pad = np.zeros((C, 64), np.float32)
        wqk = np.ascontiguousarray(np.concatenate(
            [q[:, 0:128], k[:, 0:128], q[:, 128:192], pad, k[:, 128:192], pad],
            axis=1))
        wv = np.zeros((C, 256), np.float32)
        wv[:, 0:192] = W_qkv[:, 2 * C + hg * 192: 2 * C + (hg + 1) * 192]
        wp = np.ascontiguousarray(
            W_proj[hg * 192:(hg + 1) * 192, :]).astype(ml_dtypes.bfloat16)
        in_maps.append({"xT": xT, "wqk": wqk, "wv": wv, "wp": wp})
    return in_maps


def unshard(results):
    y = np.zeros((B, T, C), np.float64)
    for core in range(NCORES):
        b = core // 4
        y[b] += results[core]["yp"].astype(np.float64)
    return y.astype(np.float32)


# ---------------- PJRT runner (axon-tunneled NeuronCores) ----------------

_RUNNERS = {}


def get_runner(reps: int = 1):
    """Build (once) and return fn(in_maps) -> list[dict] for 8 cores."""
    if reps in _RUNNERS:
        return _RUNNERS[reps]

    import jax
    from jax.sharding import Mesh, PartitionSpec, NamedSharding
    from jax.experimental.shard_map import shard_map
    from concourse.bass2jax import (
        _bass_exec_p, install_neuronx_cc_hook, partition_id_tensor)

    nc = build_program(reps=reps)
    install_neuronx_cc_hook()

    partition_name = nc.partition_id_tensor.name if nc.partition_id_tensor else None
    in_names, out_names, out_avals = [], [], []
    for alloc in nc.m.functions[0].allocations:
        if not isinstance(alloc, mybir.MemoryLocationSet):
            continue
        name = alloc.memorylocations[0].name
        if alloc.kind == "ExternalInput":
            if name != partition_name:
                in_names.append(name)
        elif alloc.kind == "ExternalOutput":
            out_names.append(name)
            out_avals.append(jax.core.ShapedArray(
                tuple(alloc.tensor_shape), mybir.dt.np(alloc.dtype)))
    n_params = len(in_names)
    all_in_names = in_names + out_names + ([partition_name] if partition_name else [])

    def _body(*args):
        operands = list(args)
        if partition_name is not None:
            operands.append(partition_id_tensor())
        outs = _bass_exec_p.bind(
            *operands, out_avals=tuple(out_avals), in_names=tuple(all_in_names),
            out_names=tuple(out_names), lowering_input_output_aliases=(),
            sim_require_finite=True, sim_require_nnan=True, nc=nc)
        return tuple(outs)

    devices = jax.devices()[:NCORES]
    mesh = Mesh(np.asarray(devices), ("core",))
    spec = (PartitionSpec("core"),)
    fn = jax.jit(
        shard_map(_body, mesh=mesh, in_specs=spec * (n_params + len(out_names)),
                  out_specs=spec * len(out_names), check_rep=False),
        donate_argnums=tuple(range(n_params, n_params + len(out_names))),
        keep_unused=True)
    sharding = NamedSharding(mesh, PartitionSpec("core"))

    def run(in_maps, in_dev=None):
        if in_dev is None:
            in_dev = put_inputs(in_maps, sharding, in_names)
        zeros = [
            jax.device_put(
                np.zeros((NCORES * a.shape[0], *a.shape[1:]), a.dtype), sharding)
            for a in out_avals]
        outs = fn(*in_dev, *zeros)
        return [
            {name: np.asarray(outs[i]).reshape(NCORES, *out_avals[i].shape)[c]
             for i, name in enumerate(out_names)}
            for c in range(NCORES)]

    def put_inputs(in_maps, sharding_=None, names=None):
        import jax as _jax
        sh = sharding_ or sharding
        nm = names or in_names
        return [
            _jax.device_put(
                np.concatenate([in_maps[c][n] for c in range(NCORES)], axis=0), sh)
            for n in nm]

    run.put_inputs = put_inputs
    run.sharding = sharding
    run.in_names = in_names
    _RUNNERS[reps] = run
    return run


def kernel(x, W_qkv, W_proj):
    x = np.asarray(x, dtype=np.float32)
    W_qkv = np.asarray(W_qkv, dtype=np.float32)
    W_proj = np.asarray(W_proj, dtype=np.float32)
    run = get_runner(reps=1)
    results = run(shard_inputs(x, W_qkv, W_proj))
    return unshard(results)
